# revision 10
# baseline (speedup 1.0000x reference)
"""FBPINN forward kernel for Trainium2 (8 NeuronCores), MoE-routing style.

Strategy
--------
The reference evaluates all S=64 subdomain MLPs densely on all N=131072
points, then combines with a sigmoid-product window w_s(x) normalized over
S.  The window decays like exp(-s_x * d) with s_x ~ 4266 beyond each
subdomain's core cell, so each point has non-negligible w for at most 2
subdomains.  We route points to subdomains on the host (exact interval
test: every dropped (s, point) pair has window sigmoid args <= -TAU, i.e.
relative window weight < e^-TAU, far below the rel-err budget), pad each
subdomain's point list to a common PAD, and evaluate the tiny MLPs on
device, expert-parallel: 8 subdomains per core, packed 4-at-a-time into
block-diagonal 128-wide matmuls.

Device pipeline (v2):
- All matmuls run as float32r (TF32-like single-pass: 1 PE cycle/column
  for >=256-column tiles vs 4 for fp32; measured ~2.8e-3 rel err, well
  inside the 2e-2 budget).
- Activations cover 1024-column megatiles spanning two PSUM banks,
  amortizing the ~220ns per-instruction access latency of the Act engine.
- The two 4-subdomain groups are interleaved stage-by-stage so the Act
  engine (the post-fp32r bottleneck at ~0.833ns/col and 3 tanh layers)
  never waits on the PE's layer-to-layer dependency chain.
- PSUM (8 banks): per group, tag A (2 banks) holds p1 then p3, tag B
  (2 banks) holds p2 then the 4-row out-projection; out rows are
  evacuated PSUM->SBUF by the otherwise-idle Vector engine and DMAed
  out per megatile.
- A tiny wi-only DMA precedes the bulk weight blob so the first matmul
  only waits on ~5KB, and a dummy 1-col act pulls the tanh ACT_TABLE_LOAD
  to the top of the Scalar stream where it overlaps framework startup.
Host does: routing, window weights, scatter-add normalization, boundary
condition. Cross-subdomain reduction happens in the host scatter-add, so
no collectives are needed.
"""

import numpy as np
from contextlib import ExitStack

S = 64
N_DIM = 2
H = 32
SCALE, SHIFT = 1.0, 0.0
NCORES = 8
SUB_PER_CORE = S // NCORES      # 8
G = 2                           # groups of 4 subdomains per core
TAU = 12.0                      # dropped window weight < e^-TAU of scale
MT = 1024                       # megatile columns (act granularity, 2 PSUM banks)
T = 512                         # matmul column chunk (1 PSUM bank)

_BUILD_CACHE = {}


def _chunks(total, step):
    out, off = [], 0
    while off < total:
        out.append((off, min(step, total - off)))
        off += step
    return out


def _build_bass(pad):
    import concourse.bass as bass
    import concourse.tile as tile
    from concourse import bacc, mybir

    f32 = mybir.dt.float32
    f32r = mybir.dt.float32r
    bf16 = mybir.dt.bfloat16
    nc = bacc.Bacc("TRN2", target_bir_lowering=False, debug=False,
                   num_devices=NCORES)
    xb = nc.dram_tensor("xb", [G, 9, pad], f32r, kind="ExternalInput").ap()
    wb = nc.dram_tensor("wb", [G, 128, 130], f32r, kind="ExternalInput").ap()
    wbh = nc.dram_tensor("wbh", [G, 128, 260], bf16, kind="ExternalInput").ap()
    o = nc.dram_tensor("o", [G, 4, pad], f32, kind="ExternalOutput").ap()

    tanh = mybir.ActivationFunctionType.Tanh
    megas = _chunks(pad, MT)

    with tile.TileContext(nc) as tc, ExitStack() as ctx:
        consts = ctx.enter_context(tc.tile_pool(name="consts", bufs=1))
        hpool = ctx.enter_context(tc.tile_pool(name="hs", bufs=1))
        opool = ctx.enter_context(tc.tile_pool(name="os", bufs=1))
        psum = ctx.enter_context(tc.tile_pool(name="ps", bufs=1, space="PSUM"))

        xb_t, wb_t, wbh_t = {}, {}, {}
        wi_t, wh_t, bh_t, wo_t, o_sb = {}, {}, {}, {}, {}
        for g in range(G):
            xb_t[g] = consts.tile([9, pad], f32r, tag=f"xb{g}", name=f"xbt{g}")
            wb_t[g] = consts.tile([128, 130], f32r, tag=f"wb{g}", name=f"wbt{g}")
            wbh_t[g] = consts.tile([128, 260], bf16, tag=f"wbh{g}",
                                   name=f"wbht{g}")
        nc.sync.dma_start(out=wb_t[0][:], in_=wb[0])
        nc.sync.dma_start(out=xb_t[0][:], in_=xb[0])
        nc.sync.dma_start(out=wbh_t[0][:], in_=wbh[0])
        nc.scalar.dma_start(out=wb_t[1][:], in_=wb[1])
        nc.scalar.dma_start(out=wbh_t[1][:], in_=wbh[1])
        nc.scalar.dma_start(out=xb_t[1][:], in_=xb[1])
        for g in range(G):
            wi_t[g] = wb_t[g][0:9, 0:128]
            bh_t[g, 0] = wb_t[g][:, 128:129].bitcast(f32)
            bh_t[g, 1] = wb_t[g][:, 129:130].bitcast(f32)
            wh_t[g, 0] = wbh_t[g][:, 0:128]
            wh_t[g, 1] = wbh_t[g][:, 128:256]
            wo_t[g] = wbh_t[g][:, 256:260]
            o_sb[g] = opool.tile([4, pad], f32, tag=f"o{g}", name=f"osb{g}")

        # --- PSUM layout: 4 tags x 2 banks = all 8 banks.
        def ptile(tag):
            return psum.tile([128, MT], f32, tag=tag, bufs=1,
                             padded_shape=[128, MT], name=tag + "n")

        # --- warmup: ramp the PE clock (0.65 -> 2.4 GHz needs ~3us of
        # sustained activity) while framework startup and input DMAs are
        # still in flight, then absorb the g0 DMA semaphores.  The dummy
        # 1x1 act hoists the tanh ACT_TABLE_LOAD to the top of the Scalar
        # stream (bacc inserts the load right before it).
        warm = hpool.tile([128, T], f32, tag="warm", name="warm")
        nc.vector.memset(warm[:], 0.0)
        nc.scalar.activation(warm[0:1, 0:1], warm[0:1, 0:1], tanh)
        wslot = ptile("pBg1")
        for i in range(2):
            nc.tensor.matmul(wslot[0:1, 0:T], warm[:, 0:1], warm[:],
                             start=True, stop=True, skip_group_check=True)
        for i, wt in enumerate((wb_t[0], xb_t[0])):
            nc.tensor.matmul(wslot[0:1, 0:2], wt[0:1, 0:1], wt[0:1, 0:2],
                             start=(i == 0), stop=(i == 1),
                             skip_group_check=True)

        # --- main pipeline: stage-interleaved across the two groups.
        # PE stream per megatile it:
        #   po_g0(it-1), p1_g0(it), po_g1(it-1), p1_g1(it),
        #   p2_g0(it), p2_g1(it), p3_g0(it), p3_g1(it)
        # keeps the Act engine gap-free at megatile boundaries.
        pend = {}                    # g -> (pB, h3, off, tsz) awaiting out-proj

        def flush_po(g):
            if g not in pend:
                return
            pB, h3, off, tsz = pend.pop(g)
            for c, csz in _chunks(tsz, T):
                nc.tensor.matmul(pB[0:4, c:c + csz], wo_t[g],
                                 h3[:, c:c + csz],
                                 start=True, stop=True, skip_group_check=True)
            nc.vector.tensor_copy(o_sb[g][:, off:off + tsz], pB[0:4, 0:tsz])
            nc.sync.dma_start(out=o[g][:, off:off + tsz],
                              in_=o_sb[g][:, off:off + tsz])

        for off, tsz in megas:
            h1, h2, h3 = {}, {}, {}
            pA, pB = {}, {}
            for g in range(G):
                flush_po(g)
                pA[g] = ptile(f"pAg{g}")
                for c, csz in _chunks(tsz, T):
                    nc.tensor.matmul(pA[g][:, c:c + csz], wi_t[g],
                                     xb_t[g][:, off + c:off + c + csz],
                                     start=True, stop=True,
                                     skip_group_check=True)
            for g in range(G):
                h1[g] = hpool.tile([128, tsz], bf16, tag=f"h1g{g}", bufs=2,
                                   padded_shape=[128, MT], name=f"h1g{g}n")
                nc.scalar.activation(h1[g][:], pA[g][:, 0:tsz], tanh)
                pB[g] = ptile(f"pBg{g}")
                for c, csz in _chunks(tsz, T):
                    nc.tensor.matmul(pB[g][:, c:c + csz], wh_t[g, 0],
                                     h1[g][:, c:c + csz],
                                     start=True, stop=True,
                                     skip_group_check=True)
            for g in range(G):
                h2[g] = hpool.tile([128, tsz], bf16, tag=f"h2g{g}", bufs=2,
                                   padded_shape=[128, MT], name=f"h2g{g}n")
                nc.scalar.activation(h2[g][:], pB[g][:, 0:tsz], tanh,
                                     bias=bh_t[g, 0])
                pA[g] = ptile(f"pAg{g}")
                for c, csz in _chunks(tsz, T):
                    nc.tensor.matmul(pA[g][:, c:c + csz], wh_t[g, 1],
                                     h2[g][:, c:c + csz],
                                     start=True, stop=True,
                                     skip_group_check=True)
            for g in range(G):
                h3[g] = hpool.tile([128, tsz], bf16, tag=f"h3g{g}", bufs=2,
                                   padded_shape=[128, MT], name=f"h3g{g}n")
                nc.scalar.activation(h3[g][:], pA[g][:, 0:tsz], tanh,
                                     bias=bh_t[g, 1])
                pend[g] = (ptile(f"pBg{g}"), h3[g], off, tsz)
        for g in range(G):
            flush_po(g)
    nc.compile()
    return nc


def _route(x, lo_core, hi_core, swin):
    """Per-subdomain point lists: s covers p iff all window sigmoid args >= -TAU."""
    n = x.shape[0]
    pts = []
    for si in range(S):
        m = np.ones(n, dtype=bool)
        for d in range(N_DIM):
            sd = swin[si, d]
            lo, hi = lo_core[si, d], hi_core[si, d]
            if sd >= 0:
                m &= (x[:, d] >= lo - TAU / max(sd, 1e-30)) \
                    & (x[:, d] <= hi + TAU / max(sd, 1e-30))
            else:  # pathological geometry; sigmoids flip direction
                m &= (x[:, d] <= lo + TAU / max(-sd, 1e-30)) \
                    & (x[:, d] >= hi - TAU / max(-sd, 1e-30))
        pts.append(np.nonzero(m)[0])
    return pts


def _pack(x, args64, pts, pad, Wn, bn):
    """Build the per-core device input tensors."""
    import ml_dtypes
    bf16 = ml_dtypes.bfloat16
    in_maps = []
    for c in range(NCORES):
        xb = np.zeros((G, 9, pad), np.float32)
        wbv = np.zeros((G, 128, 130), np.float32)
        wbh = np.zeros((G, 128, 260), np.float32)
        wi = wbv[:, 0:9, 0:128]
        bh0 = wbv[:, :, 128]
        bh1 = wbv[:, :, 129]
        wh0 = wbh[:, :, 0:128]
        wh1 = wbh[:, :, 128:256]
        wo = wbh[:, :, 256:260]
        for g in range(G):
            for j in range(4):
                s_ = c * SUB_PER_CORE + g * 4 + j
                idx = pts[s_]
                cnt = len(idx)
                xs = x[idx]
                xb[g, 0, :] = 1.0
                xb[g, 1 + 2 * j, :cnt] = xs[:, 0]
                xb[g, 2 + 2 * j, :cnt] = xs[:, 1]
                r = slice(32 * j, 32 * j + 32)
                for d in range(N_DIM):
                    wi[g, 1 + 2 * j + d, r] = Wn[s_, :, d]
                wi[g, 0, r] = bn[s_]
                wh0[g, r, r] = args64["W_h1"][s_].T
                wh1[g, r, r] = args64["W_h2"][s_].T
                bh0[g, r] = args64["b_h1"][s_]
                bh1[g, r] = args64["b_h2"][s_]
                wo[g, r, j] = args64["W_out"][s_, 0]
        in_maps.append({"xb": xb, "wb": wbv, "wbh": wbh.astype(bf16)})
    return in_maps


def _host_reference(x, lo_core, hi_core, lo_ext, hi_ext,
                    W_in, b_in, W_h1, b_h1, W_h2, b_h2, W_out, b_out):
    """Dense fallback (numpy, chunked) for inputs without FBPINN locality."""
    center = (lo_ext + hi_ext) * 0.5
    half_w = (hi_ext - lo_ext) * 0.5
    overlap = np.maximum(hi_ext - hi_core, lo_core - lo_ext)
    width = hi_ext - lo_ext
    s = 4.0 / (2.0 * overlap * width + 1e-8)
    sigm = lambda v: 1.0 / (1.0 + np.exp(-v))
    outs = []
    for i in range(0, x.shape[0], 8192):
        xc = x[i:i + 8192].astype(np.float64)
        xn = (xc[None] - center[:, None]) / half_w[:, None]
        hh = np.tanh(np.einsum("snd,shd->snh", xn, W_in) + b_in[:, None])
        hh = np.tanh(np.einsum("snh,skh->snk", hh, W_h1) + b_h1[:, None])
        hh = np.tanh(np.einsum("snh,skh->snk", hh, W_h2) + b_h2[:, None])
        out = np.einsum("snh,soh->sno", hh, W_out) + b_out[:, None]
        out = out * SCALE + SHIFT
        left = sigm(s[:, None] * (xc[None] - lo_core[:, None]))
        right = sigm(s[:, None] * (hi_core[:, None] - xc[None]))
        w = np.prod(left * right, axis=-1, keepdims=True)
        w = w / (np.sum(w, axis=0, keepdims=True) + 1e-8)
        u = np.sum(out * w, axis=0)
        gg = -np.sin(np.pi * xc[:, 1])[:, None]
        fac = (np.tanh(xc[:, 1] + 1) * np.tanh(xc[:, 1] - 1)
               * np.tanh(xc[:, 0]))[:, None]
        outs.append((gg + fac * u).astype(np.float32))
    return np.concatenate(outs, axis=0)


def _prepare(x, args64):
    """Routing + weight folding. Returns (pts, pad, swin, Wn, bn) or None
    if the inputs lack FBPINN locality (caller should fall back to dense)."""
    lo_core64, hi_core64 = args64["lo_core"], args64["hi_core"]
    lo_ext64, hi_ext64 = args64["lo_ext"], args64["hi_ext"]
    n = x.shape[0]
    center = (lo_ext64 + hi_ext64) * 0.5
    half_w = (hi_ext64 - lo_ext64) * 0.5
    overlap = np.maximum(hi_ext64 - hi_core64, lo_core64 - lo_ext64)
    width = hi_ext64 - lo_ext64
    swin = 4.0 / (2.0 * overlap * width + 1e-8)

    pts = _route(x, lo_core64, hi_core64, swin)
    counts = np.array([len(p) for p in pts])
    if counts.sum() > 4 * n or counts.max() > max(4 * n // S, 8192):
        return None
    pad = int(max(256, -(-counts.max() // 128) * 128))

    W_in64 = args64["W_in"]                      # (S,H,D)
    Wn = W_in64 / half_w[:, None, :]             # (S,H,D)
    bn = args64["b_in"] - np.einsum("shd,sd->sh", W_in64, center / half_w)
    return pts, pad, swin, Wn, bn


def _epilogue(x, args64, pts, swin, o_by_sub):
    """Window weights + normalized scatter-add + boundary condition.
    o_by_sub: callable s -> raw device MLP outputs for subdomain s's slots."""
    n = x.shape[0]
    lo_core64, hi_core64 = args64["lo_core"], args64["hi_core"]
    b_out64 = args64["b_out"]
    numer = np.zeros(n, np.float64)
    denom = np.zeros(n, np.float64)
    sigm = lambda v: 1.0 / (1.0 + np.exp(-v))
    for s_ in range(S):
        idx = pts[s_]
        cnt = len(idx)
        if cnt == 0:
            continue
        xs = x[idx].astype(np.float64)
        arg_l = swin[s_] * (xs - lo_core64[s_])
        arg_r = swin[s_] * (hi_core64[s_] - xs)
        w = np.prod(sigm(arg_l) * sigm(arg_r), axis=-1)
        out_s = (o_by_sub(s_)[:cnt].astype(np.float64)
                 + b_out64[s_, 0]) * SCALE + SHIFT
        np.add.at(numer, idx, out_s * w)
        np.add.at(denom, idx, w)
    u = numer / (denom + 1e-8)
    x64 = x.astype(np.float64)
    gg = -np.sin(np.pi * x64[:, 1])
    fac = np.tanh(x64[:, 1] + 1.0) * np.tanh(x64[:, 1] - 1.0) * np.tanh(x64[:, 0])
    return (gg + fac * u)[:, None].astype(np.float32)


def kernel(x, lo_core, hi_core, lo_ext, hi_ext,
           W_in, b_in, W_h1, b_h1, W_h2, b_h2, W_out, b_out,
           _profile=False):
    x = np.asarray(x, np.float32)
    args64 = {k: np.asarray(v, np.float64) for k, v in dict(
        lo_core=lo_core, hi_core=hi_core, lo_ext=lo_ext, hi_ext=hi_ext,
        W_in=W_in, b_in=b_in, W_h1=W_h1, b_h1=b_h1, W_h2=W_h2, b_h2=b_h2,
        W_out=W_out, b_out=b_out).items()}

    prep = _prepare(x, args64)
    if prep is None:
        return _host_reference(x, **args64)
    pts, pad, swin, Wn, bn = prep

    in_maps = _pack(x, args64, pts, pad, Wn, bn)

    from concourse.bass_utils import run_bass_kernel_spmd
    if pad not in _BUILD_CACHE:
        _BUILD_CACHE[pad] = _build_bass(pad)
    nc = _BUILD_CACHE[pad]
    res = run_bass_kernel_spmd(nc, in_maps, list(range(NCORES)),
                               trace=bool(_profile))

    def o_by_sub(s_):
        c, rem = divmod(s_, SUB_PER_CORE)
        g, j = divmod(rem, 4)
        return res.results[c]["o"][g, j]

    final = _epilogue(x, args64, pts, swin, o_by_sub)
    if _profile:
        return final, res
    return final


# revision 11
# speedup vs baseline: 1.0924x; 1.0924x over previous
"""FBPINN forward kernel for Trainium2 (8 NeuronCores), MoE-routing style.

Strategy
--------
The reference evaluates all S=64 subdomain MLPs densely on all N=131072
points, then combines with a sigmoid-product window w_s(x) normalized over
S.  The window decays like exp(-s_x * d) with s_x ~ 4266 beyond each
subdomain's core cell, so each point has non-negligible w for at most 2
subdomains.  We route points to subdomains on the host (exact interval
test: every dropped (s, point) pair has window sigmoid args <= -TAU, i.e.
relative window weight < e^-TAU, far below the rel-err budget), pad each
subdomain's point list to a common PAD, and evaluate the tiny MLPs on
device, expert-parallel: 8 subdomains per core, packed 4-at-a-time into
block-diagonal 128-wide matmuls.

Device pipeline (v2):
- All matmuls run as float32r (TF32-like single-pass: 1 PE cycle/column
  for >=256-column tiles vs 4 for fp32; measured ~2.8e-3 rel err, well
  inside the 2e-2 budget).
- Activations cover 1024-column megatiles spanning two PSUM banks,
  amortizing the ~220ns per-instruction access latency of the Act engine.
- The two 4-subdomain groups are interleaved stage-by-stage so the Act
  engine (the post-fp32r bottleneck at ~0.833ns/col and 3 tanh layers)
  never waits on the PE's layer-to-layer dependency chain.
- PSUM (8 banks): per group, tag A (2 banks) holds p1 then p3, tag B
  (2 banks) holds p2 then the 4-row out-projection; out rows are
  evacuated PSUM->SBUF by the otherwise-idle Vector engine and DMAed
  out per megatile.
- A tiny wi-only DMA precedes the bulk weight blob so the first matmul
  only waits on ~5KB, and a dummy 1-col act pulls the tanh ACT_TABLE_LOAD
  to the top of the Scalar stream where it overlaps framework startup.
Host does: routing, window weights, scatter-add normalization, boundary
condition. Cross-subdomain reduction happens in the host scatter-add, so
no collectives are needed.
"""

import numpy as np
from contextlib import ExitStack

S = 64
N_DIM = 2
H = 32
SCALE, SHIFT = 1.0, 0.0
NCORES = 8
SUB_PER_CORE = S // NCORES      # 8
G = 2                           # groups of 4 subdomains per core
TAU = 12.0                      # dropped window weight < e^-TAU of scale
MT = 1024                       # megatile columns (act granularity, 2 PSUM banks)
T = 512                         # matmul column chunk (1 PSUM bank)

_BUILD_CACHE = {}


def _chunks(total, step):
    out, off = [], 0
    while off < total:
        out.append((off, min(step, total - off)))
        off += step
    return out


def _build_bass(pad):
    import concourse.bass as bass
    import concourse.tile as tile
    from concourse import bacc, mybir

    f32 = mybir.dt.float32
    f32r = mybir.dt.float32r
    bf16 = mybir.dt.bfloat16
    nc = bacc.Bacc("TRN2", target_bir_lowering=False, debug=False,
                   num_devices=NCORES)
    xb = nc.dram_tensor("xb", [G, 9, pad], f32r, kind="ExternalInput").ap()
    wb = nc.dram_tensor("wb", [G, 128, 130], f32r, kind="ExternalInput").ap()
    wbh = nc.dram_tensor("wbh", [G, 128, 260], bf16, kind="ExternalInput").ap()
    o = nc.dram_tensor("o", [G, 4, pad], f32, kind="ExternalOutput").ap()

    tanh = mybir.ActivationFunctionType.Tanh
    megas = _chunks(pad, MT)

    with tile.TileContext(nc) as tc, ExitStack() as ctx:
        consts = ctx.enter_context(tc.tile_pool(name="consts", bufs=1))
        hpool = ctx.enter_context(tc.tile_pool(name="hs", bufs=1))
        opool = ctx.enter_context(tc.tile_pool(name="os", bufs=1))
        psum = ctx.enter_context(tc.tile_pool(name="ps", bufs=1, space="PSUM"))

        xb_t, wb_t, wbh_t = {}, {}, {}
        wi_t, wh_t, bh_t, wo_t, o_sb = {}, {}, {}, {}, {}
        for g in range(G):
            xb_t[g] = consts.tile([9, pad], f32r, tag=f"xb{g}", name=f"xbt{g}")
            wb_t[g] = consts.tile([128, 130], f32r, tag=f"wb{g}", name=f"wbt{g}")
            wbh_t[g] = consts.tile([128, 260], bf16, tag=f"wbh{g}",
                                   name=f"wbht{g}")
        nc.sync.dma_start(out=wb_t[0][:], in_=wb[0])
        nc.sync.dma_start(out=xb_t[0][:], in_=xb[0])
        nc.sync.dma_start(out=wbh_t[0][:], in_=wbh[0])
        nc.gpsimd.dma_start(out=wb_t[1][:], in_=wb[1])
        nc.gpsimd.dma_start(out=wbh_t[1][:], in_=wbh[1])
        nc.gpsimd.dma_start(out=xb_t[1][:], in_=xb[1])
        for g in range(G):
            wi_t[g] = wb_t[g][0:9, 0:128]
            bh_t[g, 0] = wb_t[g][:, 128:129].bitcast(f32)
            bh_t[g, 1] = wb_t[g][:, 129:130].bitcast(f32)
            wh_t[g, 0] = wbh_t[g][:, 0:128]
            wh_t[g, 1] = wbh_t[g][:, 128:256]
            wo_t[g] = wbh_t[g][:, 256:260]
            o_sb[g] = opool.tile([4, pad], f32, tag=f"o{g}", name=f"osb{g}")

        # --- PSUM layout: 4 tags x 2 banks = all 8 banks.
        def ptile(tag):
            return psum.tile([128, MT], f32, tag=tag, bufs=1,
                             padded_shape=[128, MT], name=tag + "n")

        # --- warmup: ramp the PE clock (0.65 -> 2.4 GHz needs ~3us of
        # sustained activity) while framework startup and input DMAs are
        # still in flight, then absorb the g0 DMA semaphores.  The dummy
        # 1x1 act hoists the tanh ACT_TABLE_LOAD to the top of the Scalar
        # stream (bacc inserts the load right before it).
        warm = hpool.tile([128, T], f32, tag="warm", name="warm")
        nc.vector.memset(warm[:], 0.0)
        # separate scratch for the table-load act: the warm matmuls must
        # not wait behind the 1.3us ACT_TABLE_LOAD.
        scr = hpool.tile([1, 2], f32, tag="scr", name="scr")
        nc.vector.memset(scr[:], 0.0)
        nc.scalar.activation(scr[:], scr[:], tanh)
        wslot = ptile("pBg1")
        for i in range(3):
            nc.tensor.matmul(wslot[0:1, 0:T], warm[:, 0:1], warm[:],
                             start=True, stop=True, skip_group_check=True)
        for i, wt in enumerate((wb_t[0], xb_t[0])):
            nc.tensor.matmul(wslot[0:1, 0:2], wt[0:1, 0:1], wt[0:1, 0:2],
                             start=(i == 0), stop=(i == 1),
                             skip_group_check=True)

        # --- main pipeline: stage-interleaved across the two groups.
        # PE stream per megatile it:
        #   po_g0(it-1), p1_g0(it), po_g1(it-1), p1_g1(it),
        #   p2_g0(it), p2_g1(it), p3_g0(it), p3_g1(it)
        # keeps the Act engine gap-free at megatile boundaries.
        pend = {}                    # g -> (pB, h3, off, tsz) awaiting out-proj

        def flush_po(g):
            if g not in pend:
                return
            pB, h3, off, tsz = pend.pop(g)
            for c, csz in _chunks(tsz, T):
                nc.tensor.matmul(pB[0:4, c:c + csz], wo_t[g],
                                 h3[:, c:c + csz],
                                 start=True, stop=True, skip_group_check=True)
            nc.vector.tensor_copy(o_sb[g][:, off:off + tsz], pB[0:4, 0:tsz])
            nc.sync.dma_start(out=o[g][:, off:off + tsz],
                              in_=o_sb[g][:, off:off + tsz])

        for off, tsz in megas:
            h1, h2, h3 = {}, {}, {}
            pA, pB = {}, {}
            for g in range(G):
                flush_po(g)
                pA[g] = ptile(f"pAg{g}")
                for c, csz in _chunks(tsz, T):
                    nc.tensor.matmul(pA[g][:, c:c + csz], wi_t[g],
                                     xb_t[g][:, off + c:off + c + csz],
                                     start=True, stop=True,
                                     skip_group_check=True)
            for g in range(G):
                h1[g] = hpool.tile([128, tsz], bf16, tag=f"h1g{g}", bufs=2,
                                   padded_shape=[128, MT], name=f"h1g{g}n")
                nc.scalar.activation(h1[g][:], pA[g][:, 0:tsz], tanh)
                pB[g] = ptile(f"pBg{g}")
                for c, csz in _chunks(tsz, T):
                    nc.tensor.matmul(pB[g][:, c:c + csz], wh_t[g, 0],
                                     h1[g][:, c:c + csz],
                                     start=True, stop=True,
                                     skip_group_check=True)
            for g in range(G):
                h2[g] = hpool.tile([128, tsz], bf16, tag=f"h2g{g}", bufs=2,
                                   padded_shape=[128, MT], name=f"h2g{g}n")
                nc.scalar.activation(h2[g][:], pB[g][:, 0:tsz], tanh,
                                     bias=bh_t[g, 0])
                pA[g] = ptile(f"pAg{g}")
                for c, csz in _chunks(tsz, T):
                    nc.tensor.matmul(pA[g][:, c:c + csz], wh_t[g, 1],
                                     h2[g][:, c:c + csz],
                                     start=True, stop=True,
                                     skip_group_check=True)
            for g in range(G):
                h3[g] = hpool.tile([128, tsz], bf16, tag=f"h3g{g}", bufs=2,
                                   padded_shape=[128, MT], name=f"h3g{g}n")
                nc.scalar.activation(h3[g][:], pA[g][:, 0:tsz], tanh,
                                     bias=bh_t[g, 1])
                pend[g] = (ptile(f"pBg{g}"), h3[g], off, tsz)
        for g in range(G):
            flush_po(g)
    nc.compile()
    return nc


def _route(x, lo_core, hi_core, swin):
    """Per-subdomain point lists: s covers p iff all window sigmoid args >= -TAU."""
    n = x.shape[0]
    pts = []
    for si in range(S):
        m = np.ones(n, dtype=bool)
        for d in range(N_DIM):
            sd = swin[si, d]
            lo, hi = lo_core[si, d], hi_core[si, d]
            if sd >= 0:
                m &= (x[:, d] >= lo - TAU / max(sd, 1e-30)) \
                    & (x[:, d] <= hi + TAU / max(sd, 1e-30))
            else:  # pathological geometry; sigmoids flip direction
                m &= (x[:, d] <= lo + TAU / max(-sd, 1e-30)) \
                    & (x[:, d] >= hi - TAU / max(-sd, 1e-30))
        pts.append(np.nonzero(m)[0])
    return pts


def _pack(x, args64, pts, pad, Wn, bn):
    """Build the per-core device input tensors."""
    import ml_dtypes
    bf16 = ml_dtypes.bfloat16
    in_maps = []
    for c in range(NCORES):
        xb = np.zeros((G, 9, pad), np.float32)
        wbv = np.zeros((G, 128, 130), np.float32)
        wbh = np.zeros((G, 128, 260), np.float32)
        wi = wbv[:, 0:9, 0:128]
        bh0 = wbv[:, :, 128]
        bh1 = wbv[:, :, 129]
        wh0 = wbh[:, :, 0:128]
        wh1 = wbh[:, :, 128:256]
        wo = wbh[:, :, 256:260]
        for g in range(G):
            for j in range(4):
                s_ = c * SUB_PER_CORE + g * 4 + j
                idx = pts[s_]
                cnt = len(idx)
                xs = x[idx]
                xb[g, 0, :] = 1.0
                xb[g, 1 + 2 * j, :cnt] = xs[:, 0]
                xb[g, 2 + 2 * j, :cnt] = xs[:, 1]
                r = slice(32 * j, 32 * j + 32)
                for d in range(N_DIM):
                    wi[g, 1 + 2 * j + d, r] = Wn[s_, :, d]
                wi[g, 0, r] = bn[s_]
                wh0[g, r, r] = args64["W_h1"][s_].T
                wh1[g, r, r] = args64["W_h2"][s_].T
                bh0[g, r] = args64["b_h1"][s_]
                bh1[g, r] = args64["b_h2"][s_]
                wo[g, r, j] = args64["W_out"][s_, 0]
        in_maps.append({"xb": xb, "wb": wbv, "wbh": wbh.astype(bf16)})
    return in_maps


def _host_reference(x, lo_core, hi_core, lo_ext, hi_ext,
                    W_in, b_in, W_h1, b_h1, W_h2, b_h2, W_out, b_out):
    """Dense fallback (numpy, chunked) for inputs without FBPINN locality."""
    center = (lo_ext + hi_ext) * 0.5
    half_w = (hi_ext - lo_ext) * 0.5
    overlap = np.maximum(hi_ext - hi_core, lo_core - lo_ext)
    width = hi_ext - lo_ext
    s = 4.0 / (2.0 * overlap * width + 1e-8)
    sigm = lambda v: 1.0 / (1.0 + np.exp(-v))
    outs = []
    for i in range(0, x.shape[0], 8192):
        xc = x[i:i + 8192].astype(np.float64)
        xn = (xc[None] - center[:, None]) / half_w[:, None]
        hh = np.tanh(np.einsum("snd,shd->snh", xn, W_in) + b_in[:, None])
        hh = np.tanh(np.einsum("snh,skh->snk", hh, W_h1) + b_h1[:, None])
        hh = np.tanh(np.einsum("snh,skh->snk", hh, W_h2) + b_h2[:, None])
        out = np.einsum("snh,soh->sno", hh, W_out) + b_out[:, None]
        out = out * SCALE + SHIFT
        left = sigm(s[:, None] * (xc[None] - lo_core[:, None]))
        right = sigm(s[:, None] * (hi_core[:, None] - xc[None]))
        w = np.prod(left * right, axis=-1, keepdims=True)
        w = w / (np.sum(w, axis=0, keepdims=True) + 1e-8)
        u = np.sum(out * w, axis=0)
        gg = -np.sin(np.pi * xc[:, 1])[:, None]
        fac = (np.tanh(xc[:, 1] + 1) * np.tanh(xc[:, 1] - 1)
               * np.tanh(xc[:, 0]))[:, None]
        outs.append((gg + fac * u).astype(np.float32))
    return np.concatenate(outs, axis=0)


def _prepare(x, args64):
    """Routing + weight folding. Returns (pts, pad, swin, Wn, bn) or None
    if the inputs lack FBPINN locality (caller should fall back to dense)."""
    lo_core64, hi_core64 = args64["lo_core"], args64["hi_core"]
    lo_ext64, hi_ext64 = args64["lo_ext"], args64["hi_ext"]
    n = x.shape[0]
    center = (lo_ext64 + hi_ext64) * 0.5
    half_w = (hi_ext64 - lo_ext64) * 0.5
    overlap = np.maximum(hi_ext64 - hi_core64, lo_core64 - lo_ext64)
    width = hi_ext64 - lo_ext64
    swin = 4.0 / (2.0 * overlap * width + 1e-8)

    pts = _route(x, lo_core64, hi_core64, swin)
    counts = np.array([len(p) for p in pts])
    if counts.sum() > 4 * n or counts.max() > max(4 * n // S, 8192):
        return None
    pad = int(max(256, -(-counts.max() // 128) * 128))

    W_in64 = args64["W_in"]                      # (S,H,D)
    Wn = W_in64 / half_w[:, None, :]             # (S,H,D)
    bn = args64["b_in"] - np.einsum("shd,sd->sh", W_in64, center / half_w)
    return pts, pad, swin, Wn, bn


def _epilogue(x, args64, pts, swin, o_by_sub):
    """Window weights + normalized scatter-add + boundary condition.
    o_by_sub: callable s -> raw device MLP outputs for subdomain s's slots."""
    n = x.shape[0]
    lo_core64, hi_core64 = args64["lo_core"], args64["hi_core"]
    b_out64 = args64["b_out"]
    numer = np.zeros(n, np.float64)
    denom = np.zeros(n, np.float64)
    sigm = lambda v: 1.0 / (1.0 + np.exp(-v))
    for s_ in range(S):
        idx = pts[s_]
        cnt = len(idx)
        if cnt == 0:
            continue
        xs = x[idx].astype(np.float64)
        arg_l = swin[s_] * (xs - lo_core64[s_])
        arg_r = swin[s_] * (hi_core64[s_] - xs)
        w = np.prod(sigm(arg_l) * sigm(arg_r), axis=-1)
        out_s = (o_by_sub(s_)[:cnt].astype(np.float64)
                 + b_out64[s_, 0]) * SCALE + SHIFT
        np.add.at(numer, idx, out_s * w)
        np.add.at(denom, idx, w)
    u = numer / (denom + 1e-8)
    x64 = x.astype(np.float64)
    gg = -np.sin(np.pi * x64[:, 1])
    fac = np.tanh(x64[:, 1] + 1.0) * np.tanh(x64[:, 1] - 1.0) * np.tanh(x64[:, 0])
    return (gg + fac * u)[:, None].astype(np.float32)


def kernel(x, lo_core, hi_core, lo_ext, hi_ext,
           W_in, b_in, W_h1, b_h1, W_h2, b_h2, W_out, b_out,
           _profile=False):
    x = np.asarray(x, np.float32)
    args64 = {k: np.asarray(v, np.float64) for k, v in dict(
        lo_core=lo_core, hi_core=hi_core, lo_ext=lo_ext, hi_ext=hi_ext,
        W_in=W_in, b_in=b_in, W_h1=W_h1, b_h1=b_h1, W_h2=W_h2, b_h2=b_h2,
        W_out=W_out, b_out=b_out).items()}

    prep = _prepare(x, args64)
    if prep is None:
        return _host_reference(x, **args64)
    pts, pad, swin, Wn, bn = prep

    in_maps = _pack(x, args64, pts, pad, Wn, bn)

    from concourse.bass_utils import run_bass_kernel_spmd
    if pad not in _BUILD_CACHE:
        _BUILD_CACHE[pad] = _build_bass(pad)
    nc = _BUILD_CACHE[pad]
    res = run_bass_kernel_spmd(nc, in_maps, list(range(NCORES)),
                               trace=bool(_profile))

    def o_by_sub(s_):
        c, rem = divmod(s_, SUB_PER_CORE)
        g, j = divmod(rem, 4)
        return res.results[c]["o"][g, j]

    final = _epilogue(x, args64, pts, swin, o_by_sub)
    if _profile:
        return final, res
    return final
